# revision 35
# baseline (speedup 1.0000x reference)
"""TRN2 Bass kernel for nn_AttentionBlock (GroupNorm32 + 8-head attention + proj + residual).

Sharding: data-parallel over batch — batch=8, one batch element per NeuronCore, no
collectives. Per core: GroupNorm stats via DVE reductions + tiny mask matmuls,
qkv/attention/proj as bf16 matmuls on TensorE (scores packed 2-heads-per-array via
row groups), softmax exp on ScalarE (single natural_log_exp table set, rsqrt done as
exp(-0.5*ln v)), un-normalized attention accumulated with an appended ones-row in vT
so the softmax denominator falls out of the same matmuls, division applied via a tiny
mask matmul broadcast + DVE multiply.

Self-contained: hardcodes shapes from the problem spec (x (8,512,32,32) f32 etc).
"""
import numpy as np
import ml_dtypes

B, CH, HH, WW = 8, 512, 32, 32
L = HH * WW                  # 1024
HEADS = 8
GROUPS = 32
EPS = 1e-5
DH = CH // HEADS             # 64
KC = CH // 128               # 4 c-chunks
OC3 = 3 * CH // 128          # 12 qkv o-chunks
SC = L // 128                # 8 s/l-chunks
TC = L // 512                # 2 t-chunks
GN_N = (CH // GROUPS) * L    # elements per group = 16384

_cache = {}


def _build(has_qkv_bias, has_proj_bias, debug=False):
    import concourse.bass as bass
    import concourse.tile as tile
    from concourse import bacc, mybir
    import bass_rust as _bass_rust
    from concourse.hw_specs import get_activation_tables

    F32 = mybir.dt.float32
    BF16 = mybir.dt.bfloat16
    AF = mybir.ActivationFunctionType
    OP = mybir.AluOpType
    AX = mybir.AxisListType

    class _Bacc(bacc.Bacc):
        # Pin Exp/Ln to the combined `natural_log_exp_and_others` table set so
        # alternating Ln/Exp activations don't thrash ACT_TABLE_LOADs (~2.7us
        # each). Same algorithm as Bacc.insert_act_table_loads, with Exp/Ln
        # stripped from every other set so the chooser can't pick them.
        def insert_act_table_loads(self):
            has_activation = any(
                isinstance(i, mybir.InstActivation)
                for b in self.main_func.blocks
                for i in b.instructions
            )
            if not has_activation:
                return
            combo = {AF.Exp, AF.Ln}
            tables = []
            for name, fns in get_activation_tables(self.m.arch).items():
                if name != "natural_log_exp_and_others":
                    fns = {f for f in fns if f not in combo}
                tables.append((name, fns))
            _bass_rust.insert_act_table_loads(self, tables)

    nc = _Bacc("TRN2", target_bir_lowering=False, debug=False, num_devices=8)

    x_d = nc.dram_tensor("x", [CH, L], F32, kind="ExternalInput").ap()
    qw_d = nc.dram_tensor("qkv_wt", [CH, 3 * CH], BF16, kind="ExternalInput").ap()
    pw_d = nc.dram_tensor("proj_wt", [CH, CH], BF16, kind="ExternalInput").ap()
    gmask_d = nc.dram_tensor("gmask", [128, 8], F32, kind="ExternalInput").ap()
    gmaskT_d = nc.dram_tensor("gmask_t", [8, 128], F32, kind="ExternalInput").ap()
    if has_qkv_bias:
        qkb_d = nc.dram_tensor("qk_bias", [128, 8], F32, kind="ExternalInput").ap()
        vb_d = nc.dram_tensor("v_bias", [128, KC], F32, kind="ExternalInput").ap()
    if has_proj_bias:
        pb_d = nc.dram_tensor("p_bias", [128, KC], F32, kind="ExternalInput").ap()
    out_d = nc.dram_tensor("out", [CH, L], F32, kind="ExternalOutput").ap()
    if debug:
        dbg = {
            "d_xhat": nc.dram_tensor("d_xhat", [128, KC * L], F32, kind="ExternalOutput").ap(),
            "d_qk": nc.dram_tensor("d_qk", [128, 8 * L], F32, kind="ExternalOutput").ap(),
            "d_vt": nc.dram_tensor("d_vt", [128, SC * HEADS * 65], F32, kind="ExternalOutput").ap(),
            "d_asb": nc.dram_tensor("d_asb", [128, KC * L], F32, kind="ExternalOutput").ap(),
            "d_ew0": nc.dram_tensor("d_ew0", [128, L], F32, kind="ExternalOutput").ap(),
        }

    with tile.TileContext(nc) as tc:
        import contextlib
        ctx = contextlib.ExitStack()
        pers = ctx.enter_context(tc.tile_pool(name="pers", bufs=1))
        scr = ctx.enter_context(tc.tile_pool(name="scr", bufs=2))
        ewp = ctx.enter_context(tc.tile_pool(name="ewp", bufs=6))
        dvp = ctx.enter_context(tc.tile_pool(name="dvp", bufs=2))
        asg = ctx.enter_context(tc.tile_pool(name="asg", bufs=8))
        outp = ctx.enter_context(tc.tile_pool(name="outp", bufs=3))

        # ---- PE warmup: keep HAM at K=8/8 through the stats/DMA startup chain ----
        with tc.tile_pool(name="psW", bufs=1, space="PSUM") as psW:
            wsrc = scr.tile([128, 640], BF16, tag="wsrc")
            nc.gpsimd.memset(wsrc[:], 0.0)
            wps = psW.tile([128, 512], F32, tag="warm")
            for _ in range(64):
                nc.tensor.matmul(wps[:], wsrc[:, 0:128], wsrc[:, 128:640],
                                 start=True, stop=True)

        # ---- load inputs ----
        xs = pers.tile([128, KC * L], F32, tag="xs")
        for k in range(KC):
            nc.sync.dma_start(xs[:, k * L:(k + 1) * L], x_d[128 * k:128 * (k + 1), :])
        qw = pers.tile([128, KC * 3 * CH], BF16, tag="qw")
        for k in range(KC):
            nc.sync.dma_start(qw[:, k * 3 * CH:(k + 1) * 3 * CH],
                              qw_d[128 * k:128 * (k + 1), :])
        pw = pers.tile([128, KC * CH], BF16, tag="pw")
        for k in range(KC):
            nc.sync.dma_start(pw[:, k * CH:(k + 1) * CH], pw_d[128 * k:128 * (k + 1), :])
        gmask = pers.tile([128, 8], F32, tag="gmask")
        nc.sync.dma_start(gmask[:], gmask_d[:])
        gmaskT = pers.tile([8, 128], F32, tag="gmask_t")
        nc.sync.dma_start(gmaskT[:], gmaskT_d[:])
        if has_qkv_bias:
            qkb = pers.tile([128, 8], F32, tag="qkb")
            nc.sync.dma_start(qkb[:], qkb_d[:])
            vb = pers.tile([128, KC], F32, tag="vb")
            nc.sync.dma_start(vb[:], vb_d[:])
        if has_proj_bias:
            pb = pers.tile([128, KC], F32, tag="pb")
            nc.sync.dma_start(pb[:], pb_d[:])

        # ---- GroupNorm statistics + xhat + qkv (own PSUM pool, closed after) ----
        qkv_psum = tc.tile_pool(name="psQ", bufs=4, space="PSUM")
        psQ = qkv_psum.__enter__()
        stat = pers.tile([128, 8], F32, tag="stat")   # cols 0-3 sum(x), 4-7 sum(x^2)
        for k in range(KC):
            xk = xs[:, k * L:(k + 1) * L]
            nc.vector.reduce_sum(stat[:, k:k + 1], xk, axis=AX.X)
            sq = scr.tile([128, L], F32, tag="sq")
            nc.gpsimd.tensor_mul(sq[:], xk, xk)
            nc.vector.reduce_sum(stat[:, 4 + k:5 + k], sq[:], axis=AX.X)
        gst_ps = psQ.tile([8, 8], F32, tag="ps")
        nc.tensor.matmul(gst_ps[:], gmask[:], stat[:], start=True, stop=True)
        gst = pers.tile([8, 8], F32, tag="gst")
        nc.vector.tensor_copy(gst[:], gst_ps[:])
        # s2: cols 0-3 mean(chunk), cols 4-7 rstd(chunk)
        s2 = pers.tile([8, 8], F32, tag="s2")
        var = pers.tile([8, 4], F32, tag="var")
        epsb = pers.tile([8, 1], F32, tag="epsb")
        nc.gpsimd.memset(epsb[:], EPS)
        nc.vector.tensor_scalar_mul(s2[:, 0:4], gst[:, 0:4], 1.0 / GN_N)
        nc.vector.tensor_scalar_mul(var[:], gst[:, 4:8], 1.0 / GN_N)
        # var = E[x^2] - mean^2
        msq = pers.tile([8, 4], F32, tag="msq")
        nc.vector.tensor_mul(msq[:], s2[:, 0:4], s2[:, 0:4])
        nc.vector.tensor_sub(var[:], var[:], msq[:])
        lnv = pers.tile([8, 4], F32, tag="lnv")
        nc.scalar.activation(lnv[:], var[:], AF.Ln, bias=epsb[:])
        nc.scalar.activation(s2[:, 4:8], lnv[:], AF.Exp, scale=-0.5)
        bc_ps = psQ.tile([128, 8], F32, tag="ps")
        nc.tensor.matmul(bc_ps[:], gmaskT[:], s2[:], start=True, stop=True)
        bc = pers.tile([128, 8], F32, tag="bc")   # cols 0-3 mean, 4-7 rstd per chunk
        nc.vector.tensor_copy(bc[:], bc_ps[:])

        # ---- xhat (bf16) ----
        xhat = pers.tile([128, KC * L], BF16, tag="xhat")
        for k in range(KC):
            nc.vector.tensor_scalar(
                out=xhat[:, k * L:(k + 1) * L], in0=xs[:, k * L:(k + 1) * L],
                scalar1=bc[:, k:k + 1], scalar2=bc[:, 4 + k:5 + k],
                op0=OP.subtract, op1=OP.mult)

        if debug:
            def dump_bf16(dram_ap, sb_ap, width):
                for off in range(0, width, 512):
                    w = min(512, width - off)
                    stg = outp.tile([128, 512], F32, tag="dstg")
                    nc.vector.tensor_copy(stg[:sb_ap.shape[0], :w],
                                          sb_ap[:, off:off + w])
                    nc.sync.dma_start(dram_ap[:sb_ap.shape[0], off:off + w],
                                      stg[:sb_ap.shape[0], :w])
            dump_bf16(dbg["d_xhat"], xhat[:], KC * L)

        # ---- qkv: q,k in (o, l) layout; v transposed to (l, vc) with ones column ----
        qk = pers.tile([128, 8 * L], BF16, tag="qk")   # o-chunk j: cols j*L..; j=0-3 q, 4-7 k
        with tc.spectator_scope("qkv"):
            for j in range(8):
                for t in range(TC):
                    ps = psQ.tile([128, 512], F32, tag="ps")
                    for k in range(KC):
                        nc.tensor.matmul(
                            ps[:], qw[:, k * 3 * CH + 128 * j:k * 3 * CH + 128 * (j + 1)],
                            xhat[:, k * L + 512 * t:k * L + 512 * (t + 1)],
                            start=(k == 0), stop=(k == KC - 1))
                    dst = qk[:, j * L + 512 * t:j * L + 512 * (t + 1)]
                    if has_qkv_bias:
                        nc.vector.tensor_scalar_add(dst, ps[:], qkb[:, j:j + 1])
                    else:
                        nc.vector.tensor_copy(dst, ps[:])

            vt = pers.tile([128, SC * (HEADS * 65)], BF16, tag="vt")
            for lc in range(SC):
                v3 = vt[:, lc * 520:(lc + 1) * 520].rearrange("p (h c) -> p h c", c=65)
                nc.gpsimd.memset(v3[:, :, 64:65], 1.0)
            for lc in range(SC):
                ps = psQ.tile([128, 512], F32, tag="ps")
                for k in range(KC):
                    nc.tensor.matmul(
                        ps[:], xhat[:, k * L + 128 * lc:k * L + 128 * (lc + 1)],
                        qw[:, k * 3 * CH + 2 * CH:k * 3 * CH + 3 * CH],
                        start=(k == 0), stop=(k == KC - 1))
                v3 = vt[:, lc * 520:(lc + 1) * 520].rearrange("p (h c) -> p h c", c=65)
                src = ps[:].rearrange("p (h c) -> p h c", c=64)
                nc.vector.tensor_copy(v3[:, :, 0:64], src)
        qkv_psum.__exit__(None, None, None)

        if debug:
            dump_bf16(dbg["d_qk"], qk[:], 8 * L)
            dump_bf16(dbg["d_vt"], vt[:], SC * HEADS * 65)

        # ---- attention, head pairs (2m, 2m+1) packed into PE row groups ----
        a_sb = pers.tile([128, KC * L], BF16, tag="a_sb")
        attn_psum = tc.tile_pool(name="psS", bufs=2, space="PSUM")
        psS = attn_psum.__enter__()
        attn_acc = tc.tile_pool(name="psA", bufs=4, space="PSUM")
        psA = attn_acc.__enter__()

        def do_division(sg, e, t, mm_):
            # 1/den = exp(-ln(den)) on ACT, broadcast on GpSimd, multiply on DVE
            lnt = dvp.tile([1, 512], F32, tag="lnt")
            nc.scalar.activation(lnt[:], sg[64:65, :], AF.Ln)
            rden = dvp.tile([1, 512], F32, tag="rden")
            nc.scalar.activation(rden[:], lnt[:], AF.Exp, scale=-1.0)
            bsb = dvp.tile([64, 512], F32, tag="bsb")
            nc.gpsimd.partition_broadcast(bsb[:], rden[:])
            dst = a_sb[64 * e:64 * (e + 1),
                       mm_ * L + 512 * t:mm_ * L + 512 * (t + 1)]
            nc.vector.tensor_mul(dst, sg[0:64, :], bsb[:])
            if has_qkv_bias:
                nc.vector.tensor_scalar_add(
                    dst, dst, vb[64 * e:64 * (e + 1), mm_:mm_ + 1])

        pending_div = []
        for m in range(4):
            with tc.spectator_scope(f"attn{m}"):
                ps_a = [[None, None], [None, None]]
                for e in range(2):
                    for t in range(TC):
                        pa = psA.tile([65, 512], F32, tag="pa")
                        ps_a[e][t] = pa

                def q_ap(e, t):
                    return qk[64 * e:64 * (e + 1), m * L + 512 * t:m * L + 512 * (t + 1)]

                def k_ap(e, sc):
                    return qk[64 * e:64 * (e + 1),
                              (4 + m) * L + 128 * sc:(4 + m) * L + 128 * (sc + 1)]

                def attn_mm(sc, e):
                    ew = ew_tiles[(sc, e)]
                    for t in range(TC):
                        nc.tensor.matmul(
                            ps_a[e][t][:],
                            vt[:, sc * 520 + (2 * m + e) * 65:
                               sc * 520 + (2 * m + e) * 65 + 65],
                            ew[:, 512 * t:512 * (t + 1)],
                            start=(sc == 0), stop=(sc == SC - 1))

                ew_tiles = {}
                for sc in range(SC):
                    ps_w = [None, None]
                    for e in range(2):
                        pw_t = psS.tile([128, 1024], F32, tag="ps")
                        ps_w[e] = pw_t
                    # packed score MM pairs (head 2m rows 0-63, head 2m+1 rows 64-127)
                    for t in range(TC):
                        for e in range(2):
                            nc.tensor.matmul(ps_w[e][:, 512 * t:512 * (t + 1)],
                                             k_ap(e, sc), q_ap(e, t),
                                             start=True, stop=True)
                    for e in range(2):
                        ew = ewp.tile([128, L], BF16, tag="ew")
                        ew_tiles[(sc, e)] = ew
                        nc.scalar.activation(ew[:], ps_w[e][:], AF.Exp)
                    if debug and m == 0 and sc == 0:
                        dump_bf16(dbg["d_ew0"], ew_tiles[(0, 0)][:], L)
                    # previous pair's division, one step per sc to spread ACT load
                    if pending_div:
                        do_division(*pending_div.pop(0))
                    # software-pipeline: attn MMs for sc-1 after scores for sc
                    if sc > 0:
                        for e in range(2):
                            attn_mm(sc - 1, e)
                for e in range(2):
                    attn_mm(SC - 1, e)

                # stage accumulators to SBUF so the PSUM banks free up for the
                # next head pair; the divisions run interleaved with the NEXT
                # pair's exp stream (pending_div) to avoid an ACT lump here.
                for e in range(2):
                    for t in range(TC):
                        sg = asg.tile([65, 512], F32, tag="astg")
                        nc.vector.tensor_copy(sg[:], ps_a[e][t][:])
                        pending_div.append((sg, e, t, m))
        while pending_div:
            do_division(*pending_div.pop(0))
        attn_acc.__exit__(None, None, None)
        attn_psum.__exit__(None, None, None)

        if debug:
            dump_bf16(dbg["d_asb"], a_sb[:], KC * L)

        # ---- proj + residual ----
        with tc.tile_pool(name="psP", bufs=3, space="PSUM") as psP, \
             tc.spectator_scope("proj"):
            for i in range(KC):
                for t in range(TC):
                    ps = psP.tile([128, 512], F32, tag="ps")
                    for k in range(KC):
                        nc.tensor.matmul(
                            ps[:], pw[:, k * CH + 128 * i:k * CH + 128 * (i + 1)],
                            a_sb[:, k * L + 512 * t:k * L + 512 * (t + 1)],
                            start=(k == 0), stop=(k == KC - 1))
                    ot = outp.tile([128, 512], F32, tag="ot")
                    nc.vector.tensor_add(ot[:],
                                         xs[:, i * L + 512 * t:i * L + 512 * (t + 1)],
                                         ps[:])
                    if has_proj_bias:
                        nc.vector.tensor_scalar_add(ot[:], ot[:], pb[:, i:i + 1])
                    nc.sync.dma_start(
                        out_d[128 * i:128 * (i + 1), 512 * t:512 * (t + 1)], ot[:])
        ctx.close()

    nc.compile()
    return nc


def _prep_inputs(x, norm_w, norm_b, qkv_w, qkv_b, proj_w, proj_b):
    scale = DH ** -0.25
    w_eff = (qkv_w.astype(np.float64) * norm_w.astype(np.float64)[None, :])
    b_eff = qkv_b.astype(np.float64) + w_eff @ norm_b.astype(np.float64)
    # reference splits qkv per head: row h*192 + {0:64 q, 64:128 k, 128:192 v}.
    # device layout wants [q_all_heads | k_all_heads | v_all_heads], head-major.
    perm = np.concatenate([
        np.concatenate([np.arange(h * 3 * DH + t * DH, h * 3 * DH + (t + 1) * DH)
                        for h in range(HEADS)])
        for t in range(3)])
    w_eff = w_eff[perm]
    b_eff = b_eff[perm]
    w_eff[:2 * CH] *= scale
    b_eff[:2 * CH] *= scale
    qkv_wt = np.ascontiguousarray(w_eff.T).astype(np.float32).astype(ml_dtypes.bfloat16)
    proj_wt = np.ascontiguousarray(proj_w.T).astype(ml_dtypes.bfloat16)

    p = np.arange(128)
    gmask = (p[:, None] // 16 == np.arange(8)[None, :]).astype(np.float32)
    gmask_t = np.ascontiguousarray(gmask.T)

    has_qkv_bias = bool(np.any(b_eff != 0.0))
    has_proj_bias = bool(np.any(proj_b != 0.0))
    common = {"qkv_wt": qkv_wt, "proj_wt": proj_wt, "gmask": gmask,
              "gmask_t": gmask_t}
    if has_qkv_bias:
        qk_part = b_eff[:2 * CH].astype(np.float32).reshape(8, 128).T
        v_part = b_eff[2 * CH:].astype(np.float32).reshape(KC, 128).T
        common["qk_bias"] = np.ascontiguousarray(qk_part)
        common["v_bias"] = np.ascontiguousarray(v_part)
    if has_proj_bias:
        common["p_bias"] = np.ascontiguousarray(
            proj_b.astype(np.float32).reshape(KC, 128).T)
    xf = np.ascontiguousarray(x.reshape(B, CH, L)).astype(np.float32)
    in_maps = [dict(common, x=np.ascontiguousarray(xf[i])) for i in range(B)]
    return in_maps, has_qkv_bias, has_proj_bias


def _get_nc(flags):
    if flags not in _cache:
        _cache[flags] = _build(*flags)
    return _cache[flags]


def _run(inputs, trace=False, tmpdir=None):
    from concourse.bass_utils import run_bass_kernel_spmd
    in_maps, hqb, hpb = _prep_inputs(**inputs)
    nc = _get_nc((hqb, hpb))
    kw = {}
    if trace:
        kw = dict(trace=True, tmpdir=tmpdir)
    res = run_bass_kernel_spmd(nc, in_maps, list(range(B)), **kw)
    out = np.stack([res.results[i]["out"] for i in range(B)])
    return out.reshape(B, CH, HH, WW).astype(np.float32), res


def kernel(x, norm_w, norm_b, qkv_w, qkv_b, proj_w, proj_b):
    out, _ = _run(dict(x=x, norm_w=norm_w, norm_b=norm_b, qkv_w=qkv_w,
                       qkv_b=qkv_b, proj_w=proj_w, proj_b=proj_b))
    return out


# revision 37
# speedup vs baseline: 1.0632x; 1.0632x over previous
"""TRN2 Bass kernel for nn_AttentionBlock (GroupNorm32 + 8-head attention + proj + residual).

Sharding: data-parallel over batch — batch=8, one batch element per NeuronCore, no
collectives. Per core: GroupNorm stats via DVE reductions + tiny mask matmuls,
qkv/attention/proj as bf16 matmuls on TensorE (scores packed 2-heads-per-array via
row groups), softmax exp on ScalarE (single natural_log_exp table set, rsqrt done as
exp(-0.5*ln v)), un-normalized attention accumulated with an appended ones-row in vT
so the softmax denominator falls out of the same matmuls, division applied via a tiny
mask matmul broadcast + DVE multiply.

Self-contained: hardcodes shapes from the problem spec (x (8,512,32,32) f32 etc).
"""
import numpy as np
import ml_dtypes

B, CH, HH, WW = 8, 512, 32, 32
L = HH * WW                  # 1024
HEADS = 8
GROUPS = 32
EPS = 1e-5
DH = CH // HEADS             # 64
KC = CH // 128               # 4 c-chunks
OC3 = 3 * CH // 128          # 12 qkv o-chunks
SC = L // 128                # 8 s/l-chunks
TC = L // 512                # 2 t-chunks
GN_N = (CH // GROUPS) * L    # elements per group = 16384

_cache = {}


def _build(has_qkv_bias, has_proj_bias, debug=False):
    import concourse.bass as bass
    import concourse.tile as tile
    from concourse import bacc, mybir
    import bass_rust as _bass_rust
    from concourse.hw_specs import get_activation_tables

    F32 = mybir.dt.float32
    BF16 = mybir.dt.bfloat16
    AF = mybir.ActivationFunctionType
    OP = mybir.AluOpType
    AX = mybir.AxisListType

    class _Bacc(bacc.Bacc):
        # Pin Exp/Ln to the combined `natural_log_exp_and_others` table set so
        # alternating Ln/Exp activations don't thrash ACT_TABLE_LOADs (~2.7us
        # each). Same algorithm as Bacc.insert_act_table_loads, with Exp/Ln
        # stripped from every other set so the chooser can't pick them.
        def insert_act_table_loads(self):
            has_activation = any(
                isinstance(i, mybir.InstActivation)
                for b in self.main_func.blocks
                for i in b.instructions
            )
            if not has_activation:
                return
            combo = {AF.Exp, AF.Ln}
            tables = []
            for name, fns in get_activation_tables(self.m.arch).items():
                if name != "natural_log_exp_and_others":
                    fns = {f for f in fns if f not in combo}
                tables.append((name, fns))
            _bass_rust.insert_act_table_loads(self, tables)

    nc = _Bacc("TRN2", target_bir_lowering=False, debug=False, num_devices=8)

    x_d = nc.dram_tensor("x", [CH, L], F32, kind="ExternalInput").ap()
    qw_d = nc.dram_tensor("qkv_wt", [CH, 3 * CH], BF16, kind="ExternalInput").ap()
    pw_d = nc.dram_tensor("proj_wt", [CH, CH], BF16, kind="ExternalInput").ap()
    gmask_d = nc.dram_tensor("gmask", [128, 8], F32, kind="ExternalInput").ap()
    gmaskT_d = nc.dram_tensor("gmask_t", [8, 128], F32, kind="ExternalInput").ap()
    if has_qkv_bias:
        qkb_d = nc.dram_tensor("qk_bias", [128, 8], F32, kind="ExternalInput").ap()
        vb_d = nc.dram_tensor("v_bias", [128, KC], F32, kind="ExternalInput").ap()
    if has_proj_bias:
        pb_d = nc.dram_tensor("p_bias", [128, KC], F32, kind="ExternalInput").ap()
    out_d = nc.dram_tensor("out", [CH, L], F32, kind="ExternalOutput").ap()
    if debug:
        dbg = {
            "d_xhat": nc.dram_tensor("d_xhat", [128, KC * L], F32, kind="ExternalOutput").ap(),
            "d_qk": nc.dram_tensor("d_qk", [128, 8 * L], F32, kind="ExternalOutput").ap(),
            "d_vt": nc.dram_tensor("d_vt", [128, SC * HEADS * 65], F32, kind="ExternalOutput").ap(),
            "d_asb": nc.dram_tensor("d_asb", [128, KC * L], F32, kind="ExternalOutput").ap(),
            "d_ew0": nc.dram_tensor("d_ew0", [128, L], F32, kind="ExternalOutput").ap(),
        }

    with tile.TileContext(nc) as tc:
        import contextlib
        ctx = contextlib.ExitStack()
        pers = ctx.enter_context(tc.tile_pool(name="pers", bufs=1))
        scr = ctx.enter_context(tc.tile_pool(name="scr", bufs=2))
        ewp = ctx.enter_context(tc.tile_pool(name="ewp", bufs=6))
        dvp = ctx.enter_context(tc.tile_pool(name="dvp", bufs=2))
        asg = ctx.enter_context(tc.tile_pool(name="asg", bufs=8))
        outp = ctx.enter_context(tc.tile_pool(name="outp", bufs=3))

        # ---- PE warmup: keep HAM at K=8/8 through the stats/DMA startup chain ----
        with tc.tile_pool(name="psW", bufs=1, space="PSUM") as psW:
            wsrc = scr.tile([128, 640], BF16, tag="wsrc")
            nc.gpsimd.memset(wsrc[:], 0.0)
            wps = psW.tile([128, 512], F32, tag="warm")
            for _ in range(28):
                nc.tensor.matmul(wps[:], wsrc[:, 0:128], wsrc[:, 128:640],
                                 start=True, stop=True)

        # ---- load inputs ----
        xs = pers.tile([128, KC * L], F32, tag="xs")
        for k in range(KC):
            nc.sync.dma_start(xs[:, k * L:(k + 1) * L], x_d[128 * k:128 * (k + 1), :])
        qw = pers.tile([128, KC * 3 * CH], BF16, tag="qw")
        for k in range(KC):
            nc.sync.dma_start(qw[:, k * 3 * CH:(k + 1) * 3 * CH],
                              qw_d[128 * k:128 * (k + 1), :])
        pw = pers.tile([128, KC * CH], BF16, tag="pw")
        for k in range(KC):
            nc.sync.dma_start(pw[:, k * CH:(k + 1) * CH], pw_d[128 * k:128 * (k + 1), :])
        gmask = pers.tile([128, 8], F32, tag="gmask")
        nc.sync.dma_start(gmask[:], gmask_d[:])
        gmaskT = pers.tile([8, 128], F32, tag="gmask_t")
        nc.sync.dma_start(gmaskT[:], gmaskT_d[:])
        if has_qkv_bias:
            qkb = pers.tile([128, 8], F32, tag="qkb")
            nc.sync.dma_start(qkb[:], qkb_d[:])
            vb = pers.tile([128, KC], F32, tag="vb")
            nc.sync.dma_start(vb[:], vb_d[:])
        if has_proj_bias:
            pb = pers.tile([128, KC], F32, tag="pb")
            nc.sync.dma_start(pb[:], pb_d[:])

        # ---- GroupNorm statistics + xhat, fully per-chunk so qkv can start on
        # chunk 0 while chunk 3 is still being reduced ----
        qkv_psum = tc.tile_pool(name="psQ", bufs=4, space="PSUM")
        psQ = qkv_psum.__enter__()
        epsb = pers.tile([8, 1], F32, tag="epsb")
        nc.gpsimd.memset(epsb[:], EPS)
        # trigger the (single) ACT table load off the critical path
        tldt = pers.tile([8, 1], F32, tag="tldt")
        nc.scalar.activation(tldt[:], epsb[:], AF.Exp)

        stat = pers.tile([128, 8], F32, tag="stat")  # cols 2k: sum(x), 2k+1: sum(x^2)
        xhat = pers.tile([128, KC * L], BF16, tag="xhat")
        bc = pers.tile([128, 2 * KC], F32, tag="bc")  # cols 2k mean, 2k+1 rstd
        for k in range(KC):
            xk = xs[:, k * L:(k + 1) * L]
            nc.vector.reduce_sum(stat[:, 2 * k:2 * k + 1], xk, axis=AX.X)
            sq = scr.tile([128, L], F32, tag="sq")
            nc.gpsimd.tensor_mul(sq[:], xk, xk)
            nc.vector.reduce_sum(stat[:, 2 * k + 1:2 * k + 2], sq[:], axis=AX.X)
            gst_ps = psQ.tile([8, 2], F32, tag="ps")
            nc.tensor.matmul(gst_ps[:], gmask[:], stat[:, 2 * k:2 * k + 2],
                             start=True, stop=True)
            s2k = pers.tile([8, 2], F32, tag=f"s2k{k}")   # col 0 mean, col 1 rstd
            vk = pers.tile([8, 2], F32, tag=f"vk{k}")     # col 0 var, col 1 scratch
            nc.vector.tensor_scalar_mul(s2k[:], gst_ps[:], 1.0 / GN_N)  # mean, E[x^2]
            nc.vector.tensor_mul(vk[:, 1:2], s2k[:, 0:1], s2k[:, 0:1])  # mean^2
            nc.vector.tensor_sub(vk[:, 0:1], s2k[:, 1:2], vk[:, 1:2])   # var
            nc.scalar.activation(vk[:, 1:2], vk[:, 0:1], AF.Ln, bias=epsb[:])
            nc.scalar.activation(s2k[:, 1:2], vk[:, 1:2], AF.Exp, scale=-0.5)
            bc_ps = psQ.tile([128, 2], F32, tag="ps")
            nc.tensor.matmul(bc_ps[:], gmaskT[:], s2k[:], start=True, stop=True)
            nc.vector.tensor_copy(bc[:, 2 * k:2 * k + 2], bc_ps[:])
            nc.vector.tensor_scalar(
                out=xhat[:, k * L:(k + 1) * L], in0=xk,
                scalar1=bc[:, 2 * k:2 * k + 1], scalar2=bc[:, 2 * k + 1:2 * k + 2],
                op0=OP.subtract, op1=OP.mult)

        if debug:
            def dump_bf16(dram_ap, sb_ap, width):
                for off in range(0, width, 512):
                    w = min(512, width - off)
                    stg = outp.tile([128, 512], F32, tag="dstg")
                    nc.vector.tensor_copy(stg[:sb_ap.shape[0], :w],
                                          sb_ap[:, off:off + w])
                    nc.sync.dma_start(dram_ap[:sb_ap.shape[0], off:off + w],
                                      stg[:sb_ap.shape[0], :w])
            dump_bf16(dbg["d_xhat"], xhat[:], KC * L)

        # ---- qkv: q,k in (o, l) layout; v transposed to (l, vc) with ones column ----
        qk = pers.tile([128, 8 * L], BF16, tag="qk")   # o-chunk j: cols j*L..; j=0-3 q, 4-7 k
        with tc.spectator_scope("qkv"):
            for j in range(8):
                for t in range(TC):
                    ps = psQ.tile([128, 512], F32, tag="ps")
                    for k in range(KC):
                        nc.tensor.matmul(
                            ps[:], qw[:, k * 3 * CH + 128 * j:k * 3 * CH + 128 * (j + 1)],
                            xhat[:, k * L + 512 * t:k * L + 512 * (t + 1)],
                            start=(k == 0), stop=(k == KC - 1))
                    dst = qk[:, j * L + 512 * t:j * L + 512 * (t + 1)]
                    if has_qkv_bias:
                        nc.vector.tensor_scalar_add(dst, ps[:], qkb[:, j:j + 1])
                    else:
                        nc.vector.tensor_copy(dst, ps[:])

            vt = pers.tile([128, SC * (HEADS * 65)], BF16, tag="vt")
            for lc in range(SC):
                v3 = vt[:, lc * 520:(lc + 1) * 520].rearrange("p (h c) -> p h c", c=65)
                nc.gpsimd.memset(v3[:, :, 64:65], 1.0)
            for lc in range(SC):
                ps = psQ.tile([128, 512], F32, tag="ps")
                for k in range(KC):
                    nc.tensor.matmul(
                        ps[:], xhat[:, k * L + 128 * lc:k * L + 128 * (lc + 1)],
                        qw[:, k * 3 * CH + 2 * CH:k * 3 * CH + 3 * CH],
                        start=(k == 0), stop=(k == KC - 1))
                v3 = vt[:, lc * 520:(lc + 1) * 520].rearrange("p (h c) -> p h c", c=65)
                src = ps[:].rearrange("p (h c) -> p h c", c=64)
                nc.vector.tensor_copy(v3[:, :, 0:64], src)
        qkv_psum.__exit__(None, None, None)

        if debug:
            dump_bf16(dbg["d_qk"], qk[:], 8 * L)
            dump_bf16(dbg["d_vt"], vt[:], SC * HEADS * 65)

        # ---- attention, head pairs (2m, 2m+1) packed into PE row groups ----
        a_sb = pers.tile([128, KC * L], BF16, tag="a_sb")
        attn_psum = tc.tile_pool(name="psS", bufs=2, space="PSUM")
        psS = attn_psum.__enter__()
        attn_acc = tc.tile_pool(name="psA", bufs=4, space="PSUM")
        psA = attn_acc.__enter__()

        def do_division(sg, e, t, mm_):
            # 1/den = exp(-ln(den)) on ACT, broadcast on GpSimd, multiply on DVE
            lnt = dvp.tile([1, 512], F32, tag="lnt")
            nc.scalar.activation(lnt[:], sg[64:65, :], AF.Ln)
            rden = dvp.tile([1, 512], F32, tag="rden")
            nc.scalar.activation(rden[:], lnt[:], AF.Exp, scale=-1.0)
            bsb = dvp.tile([64, 512], F32, tag="bsb")
            nc.gpsimd.partition_broadcast(bsb[:], rden[:])
            dst = a_sb[64 * e:64 * (e + 1),
                       mm_ * L + 512 * t:mm_ * L + 512 * (t + 1)]
            nc.vector.tensor_mul(dst, sg[0:64, :], bsb[:])
            if has_qkv_bias:
                nc.vector.tensor_scalar_add(
                    dst, dst, vb[64 * e:64 * (e + 1), mm_:mm_ + 1])

        pending_div = []
        for m in range(4):
            with tc.spectator_scope(f"attn{m}"):
                ps_a = [[None, None], [None, None]]
                for e in range(2):
                    for t in range(TC):
                        pa = psA.tile([65, 512], F32, tag="pa")
                        ps_a[e][t] = pa

                def q_ap(e, t):
                    return qk[64 * e:64 * (e + 1), m * L + 512 * t:m * L + 512 * (t + 1)]

                def k_ap(e, sc):
                    return qk[64 * e:64 * (e + 1),
                              (4 + m) * L + 128 * sc:(4 + m) * L + 128 * (sc + 1)]

                def attn_mm(sc, e):
                    ew = ew_tiles[(sc, e)]
                    for t in range(TC):
                        nc.tensor.matmul(
                            ps_a[e][t][:],
                            vt[:, sc * 520 + (2 * m + e) * 65:
                               sc * 520 + (2 * m + e) * 65 + 65],
                            ew[:, 512 * t:512 * (t + 1)],
                            start=(sc == 0), stop=(sc == SC - 1))

                ew_tiles = {}
                for sc in range(SC):
                    ps_w = [None, None]
                    for e in range(2):
                        pw_t = psS.tile([128, 1024], F32, tag="ps")
                        ps_w[e] = pw_t
                    # packed score MM pairs (head 2m rows 0-63, head 2m+1 rows 64-127)
                    for t in range(TC):
                        for e in range(2):
                            nc.tensor.matmul(ps_w[e][:, 512 * t:512 * (t + 1)],
                                             k_ap(e, sc), q_ap(e, t),
                                             start=True, stop=True)
                    for e in range(2):
                        ew = ewp.tile([128, L], BF16, tag="ew")
                        ew_tiles[(sc, e)] = ew
                        nc.scalar.activation(ew[:], ps_w[e][:], AF.Exp)
                    if debug and m == 0 and sc == 0:
                        dump_bf16(dbg["d_ew0"], ew_tiles[(0, 0)][:], L)
                    # previous pair's division, one step per sc to spread ACT load
                    if pending_div:
                        do_division(*pending_div.pop(0))
                    # software-pipeline: attn MMs for sc-1 after scores for sc
                    if sc > 0:
                        for e in range(2):
                            attn_mm(sc - 1, e)
                for e in range(2):
                    attn_mm(SC - 1, e)

                # stage accumulators to SBUF so the PSUM banks free up for the
                # next head pair; the divisions run interleaved with the NEXT
                # pair's exp stream (pending_div) to avoid an ACT lump here.
                for e in range(2):
                    for t in range(TC):
                        sg = asg.tile([65, 512], F32, tag="astg")
                        nc.vector.tensor_copy(sg[:], ps_a[e][t][:])
                        pending_div.append((sg, e, t, m))
        while pending_div:
            do_division(*pending_div.pop(0))
        attn_acc.__exit__(None, None, None)
        attn_psum.__exit__(None, None, None)

        if debug:
            dump_bf16(dbg["d_asb"], a_sb[:], KC * L)

        # ---- proj + residual ----
        with tc.tile_pool(name="psP", bufs=3, space="PSUM") as psP, \
             tc.spectator_scope("proj"):
            for i in range(KC):
                for t in range(TC):
                    ps = psP.tile([128, 512], F32, tag="ps")
                    for k in range(KC):
                        nc.tensor.matmul(
                            ps[:], pw[:, k * CH + 128 * i:k * CH + 128 * (i + 1)],
                            a_sb[:, k * L + 512 * t:k * L + 512 * (t + 1)],
                            start=(k == 0), stop=(k == KC - 1))
                    ot = outp.tile([128, 512], F32, tag="ot")
                    nc.vector.tensor_add(ot[:],
                                         xs[:, i * L + 512 * t:i * L + 512 * (t + 1)],
                                         ps[:])
                    if has_proj_bias:
                        nc.vector.tensor_scalar_add(ot[:], ot[:], pb[:, i:i + 1])
                    nc.sync.dma_start(
                        out_d[128 * i:128 * (i + 1), 512 * t:512 * (t + 1)], ot[:])
        ctx.close()

    nc.compile()
    return nc


def _prep_inputs(x, norm_w, norm_b, qkv_w, qkv_b, proj_w, proj_b):
    scale = DH ** -0.25
    w_eff = (qkv_w.astype(np.float64) * norm_w.astype(np.float64)[None, :])
    b_eff = qkv_b.astype(np.float64) + w_eff @ norm_b.astype(np.float64)
    # reference splits qkv per head: row h*192 + {0:64 q, 64:128 k, 128:192 v}.
    # device layout wants [q_all_heads | k_all_heads | v_all_heads], head-major.
    perm = np.concatenate([
        np.concatenate([np.arange(h * 3 * DH + t * DH, h * 3 * DH + (t + 1) * DH)
                        for h in range(HEADS)])
        for t in range(3)])
    w_eff = w_eff[perm]
    b_eff = b_eff[perm]
    w_eff[:2 * CH] *= scale
    b_eff[:2 * CH] *= scale
    qkv_wt = np.ascontiguousarray(w_eff.T).astype(np.float32).astype(ml_dtypes.bfloat16)
    proj_wt = np.ascontiguousarray(proj_w.T).astype(ml_dtypes.bfloat16)

    p = np.arange(128)
    gmask = (p[:, None] // 16 == np.arange(8)[None, :]).astype(np.float32)
    gmask_t = np.ascontiguousarray(gmask.T)

    has_qkv_bias = bool(np.any(b_eff != 0.0))
    has_proj_bias = bool(np.any(proj_b != 0.0))
    common = {"qkv_wt": qkv_wt, "proj_wt": proj_wt, "gmask": gmask,
              "gmask_t": gmask_t}
    if has_qkv_bias:
        qk_part = b_eff[:2 * CH].astype(np.float32).reshape(8, 128).T
        v_part = b_eff[2 * CH:].astype(np.float32).reshape(KC, 128).T
        common["qk_bias"] = np.ascontiguousarray(qk_part)
        common["v_bias"] = np.ascontiguousarray(v_part)
    if has_proj_bias:
        common["p_bias"] = np.ascontiguousarray(
            proj_b.astype(np.float32).reshape(KC, 128).T)
    xf = np.ascontiguousarray(x.reshape(B, CH, L)).astype(np.float32)
    in_maps = [dict(common, x=np.ascontiguousarray(xf[i])) for i in range(B)]
    return in_maps, has_qkv_bias, has_proj_bias


def _get_nc(flags):
    if flags not in _cache:
        _cache[flags] = _build(*flags)
    return _cache[flags]


def _run(inputs, trace=False, tmpdir=None):
    from concourse.bass_utils import run_bass_kernel_spmd
    in_maps, hqb, hpb = _prep_inputs(**inputs)
    nc = _get_nc((hqb, hpb))
    kw = {}
    if trace:
        kw = dict(trace=True, tmpdir=tmpdir)
    res = run_bass_kernel_spmd(nc, in_maps, list(range(B)), **kw)
    out = np.stack([res.results[i]["out"] for i in range(B)])
    return out.reshape(B, CH, HH, WW).astype(np.float32), res


def kernel(x, norm_w, norm_b, qkv_w, qkv_b, proj_w, proj_b):
    out, _ = _run(dict(x=x, norm_w=norm_w, norm_b=norm_b, qkv_w=qkv_w,
                       qkv_b=qkv_b, proj_w=proj_w, proj_b=proj_b))
    return out
